# revision 11
# baseline (speedup 1.0000x reference)
"""Trainium2 Bass kernel for nn_Attention (sparse_attention variant).

Computes, for inputs hidden/encoder_outputs [B,S,D], c_t [B,D], W [OUT,3D],
b [OUT], v [OUT]:

    cat       = concat([hidden, broadcast(c_t), encoder_outputs], axis=2)
    energy    = relu(cat @ W.T + b)            # [B, S, OUT]
    attention = energy @ v                     # [B, S]
    out       = softmax(attention, axis=1)

Strategy (8 NeuronCores, data-parallel over batch, 2 batches/core):
  - Split W = [W1 | W2 | W3] over the feature axis; the c_t term is a
    rank-1 row c2[b,:] = c_t[b] @ W2.T + b added per batch, so the big
    contraction is only 2D (hidden + encoder halves).
  - All matmuls run in fp16 with fp32 PSUM accumulate.  The contraction
    dim f must sit on SBUF partitions, so both X and W are transposed
    on-chip with SBUF->SBUF xbar DMAs (InstDmaTransposeAnt): natural
    fp32 tiles are DMA'd in (contiguous), cast to fp16 (DVE for X,
    SWDGE cast-load for W), then one batched transpose per 128-row
    tile emits the [f, 8, s] chunked-transposed layout directly.  No
    DRAM scratch round-trip and no PE transposes.
  - Startup is bank-split: the first W row-chunks (o 0..511) unblock
    bank-0 matmuls + the bank-0 half of c2 while o 512..1023 is still
    loading, so the PE starts ~25-30us in instead of waiting for W.
  - Main loop per 128-row s-tile: accumulate pre[s, o] over 16 f-chunks
    x 2 PSUM banks; VectorE adds the broadcast c2 row and does a fused
    relu(pre)*v + row-sum (accum_out) -> attention logits.
  - Softmax over S=2048 per batch: 128x16 tile, DVE free-dim reduce +
    GpSimd partition all-reduce, ScalarE exp, DVE normalize.
"""

import sys
import numpy as np

for _p in ("/opt/trn_rl_repo",):
    if _p not in sys.path:
        sys.path.insert(0, _p)

import concourse.bass as bass
import concourse.bacc as bacc
import concourse.tile as tile
from concourse import mybir, bass_isa
from concourse.bass_utils import run_bass_kernel_spmd

F32 = mybir.dt.float32
F16 = mybir.dt.float16
BF16 = mybir.dt.bfloat16
AF = mybir.ActivationFunctionType
ALU = mybir.AluOpType

B, S, D, OUT = 16, 2048, 1024, 1024
N_CORES = 8
B_LOC = B // N_CORES            # batches per core
S_LOC = B_LOC * S               # 4096 rows of X per core
N_ST = S_LOC // 128             # 32 s-tiles per core
ST_PER_B = S // 128             # 16 s-tiles per batch
FC = D // 128                   # 8 feature chunks per tensor
NB = OUT // 512                 # 2 PSUM banks across OUT
OC = OUT // 128                 # 8 row-chunks of W
PHASE_TILES = 3                 # s-tiles that run bank0 before W fully loaded


def build_nc():
    nc = bacc.Bacc("TRN2", target_bir_lowering=False, debug=False,
                   num_devices=N_CORES, dynamic_dma_scratch_size=32768)

    hid = nc.dram_tensor("hidden", [S_LOC, D], F32, kind="ExternalInput").ap()
    enc = nc.dram_tensor("enc", [S_LOC, D], F32, kind="ExternalInput").ap()
    ct = nc.dram_tensor("ct", [B_LOC, D], F32, kind="ExternalInput").ap()
    Wd = nc.dram_tensor("W", [OUT, 3 * D], F32, kind="ExternalInput").ap()
    bd = nc.dram_tensor("b", [OUT], F32, kind="ExternalInput").ap()
    vd = nc.dram_tensor("v", [OUT], F32, kind="ExternalInput").ap()
    outd = nc.dram_tensor("out", [B_LOC, S], F32, kind="ExternalOutput").ap()

    with tile.TileContext(nc) as tc:
        with (
            tc.tile_pool(name="const", bufs=1) as cpool,
            tc.tile_pool(name="wT", bufs=1) as wpool,
            tc.tile_pool(name="w16", bufs=2) as w16p,
            tc.tile_pool(name="x16", bufs=1) as x16p,
            tc.tile_pool(name="xT", bufs=1) as xTp,
            tc.tile_pool(name="scratch", bufs=2) as spool,
            tc.tile_pool(name="sm", bufs=2) as smpool,
            tc.tile_pool(name="eps", bufs=3, space=bass.MemorySpace.PSUM) as eps,
            tc.tile_pool(name="cps", bufs=1, space=bass.MemorySpace.PSUM) as cps,
        ):
            ones_k1 = cpool.tile([1, 128], F16)
            nc.vector.memset(ones_k1[:], 1.0)
            att_all = cpool.tile([128, N_ST], F32)   # attention logits

            # ---- small constants ------------------------------------------
            v_f = cpool.tile([1, OUT], F32)
            nc.scalar.dma_start(v_f[:], vd[None, :])
            v_h = cpool.tile([1, OUT], F16)
            nc.vector.tensor_copy(v_h[:], v_f[:])
            b_f = cpool.tile([1, OUT], F32)
            nc.scalar.dma_start(b_f[:], bd[None, :])
            # ctT[p, fc, bb] = c_t[bb, fc*128+p]
            ctT_f = cpool.tile([128, FC, B_LOC], F32)
            for bb in range(B_LOC):
                nc.scalar.dma_start(ctT_f[:, :, bb],
                                    ct[bb].rearrange("(fc p) -> p fc", p=128))
            ctT_h = cpool.tile([128, FC, B_LOC], F16)
            nc.vector.tensor_copy(ctT_h[:], ctT_f[:])

            # vbc[p, o] = v[o] (fp16) for the fused relu*v epilogue
            vbc_ps = cps.tile([128, OUT], F32, tag="cps")
            for ob in range(NB):
                sl = slice(ob * 512, (ob + 1) * 512)
                nc.tensor.matmul(vbc_ps[:, sl], ones_k1[:], v_h[:, sl],
                                 start=True, stop=True)
            vbc = cpool.tile([128, OUT], F16)
            nc.vector.tensor_copy(vbc[:], vbc_ps[:])

            # ---- W pipeline: SWDGE cast-load + xbar transpose -------------
            # wT[p, j, o] = W[o, j*128+p]; j in [0,8)=W1, [8,16)=W2,
            # [16,24)=W3.
            wT = wpool.tile([128, 3 * FC, OUT], F16)

            # NOTE: all xbar transposes go on the sync ring only — two
            # transposed DMAs in flight on different queues corrupt each
            # other (verified on HW).
            def emit_w(oc):
                w16 = w16p.tile([128, 3 * D], F16, tag="w16")
                nc.gpsimd.dma_start(w16[:], Wd[oc * 128:(oc + 1) * 128, :])
                nc.sync.dma_start(wT[:, :, oc * 128:(oc + 1) * 128], w16[:],
                                  transpose=True)

            # ---- X pipeline: SWDGE cast-load + xbar transpose -------------
            def emit_x(st):
                rows = slice(st * 128, (st + 1) * 128)
                x16h = x16p.tile([128, D], F16, tag="xh16", bufs=4)
                nc.gpsimd.dma_start(x16h[:], hid[rows, :])
                x16e = x16p.tile([128, D], F16, tag="xe16", bufs=4)
                nc.gpsimd.dma_start(x16e[:], enc[rows, :])
                xTh = xTp.tile([128, FC, 128], F16, tag="xTh", bufs=6)
                nc.sync.dma_start(xTh[:], x16h[:], transpose=True)
                xTe = xTp.tile([128, FC, 128], F16, tag="xTe", bufs=6)
                nc.sync.dma_start(xTe[:], x16e[:], transpose=True)
                return xTh, xTe

            # Interleave W chunks and early X tiles on the DMA rings.
            emit_w(0)
            emit_w(1)
            xts = {}
            xts[0] = emit_x(0)
            emit_w(2)
            xts[1] = emit_x(1)
            emit_w(3)
            xts[2] = emit_x(2)

            def emit_mm(e_ps, ob, xTh, xTe):
                sl = slice(ob * 512, (ob + 1) * 512)
                for fc in range(FC):
                    nc.tensor.matmul(e_ps[:, sl], xTh[:, fc, :],
                                     wT[:, fc, sl],
                                     start=(fc == 0), stop=False)
                for fc in range(FC):
                    nc.tensor.matmul(e_ps[:, sl], xTe[:, fc, :],
                                     wT[:, 2 * FC + fc, sl],
                                     start=False, stop=(fc == FC - 1))

            # c2[b, o] = c_t[b] @ W2.T + b, one PSUM row per batch
            c2bc_sb = {}

            def emit_c2_bank(c2_ps, bb, ob):
                sl = slice(ob * 512, (ob + 1) * 512)
                for fc in range(FC):
                    nc.tensor.matmul(c2_ps[:1, sl], ctT_h[:, fc, bb:bb + 1],
                                     wT[:, FC + fc, sl],
                                     start=(fc == 0), stop=(fc == FC - 1))

            def finish_c2(c2_ps, bb):
                c2b = cpool.tile([1, OUT], F16, tag=f"c2b_{bb}")
                nc.vector.tensor_add(c2b[:], c2_ps[:1, :], b_f[:])
                c2bc_ps = cps.tile([128, OUT], F32, tag="cps")
                for ob in range(NB):
                    sl = slice(ob * 512, (ob + 1) * 512)
                    nc.tensor.matmul(c2bc_ps[:, sl], ones_k1[:],
                                     c2b[:, sl], start=True, stop=True)
                c2bc = cpool.tile([128, OUT], F16, tag=f"c2bc_{bb}")
                nc.vector.tensor_copy(c2bc[:], c2bc_ps[:])
                c2bc_sb[bb] = c2bc

            # --- phase A: bank 0 only (W rows 0..511 suffice) --------------
            c2_ps0 = cps.tile([128, OUT], F32, tag="cps")
            emit_c2_bank(c2_ps0, 0, 0)
            e_tiles = {}
            for st in range(PHASE_TILES):
                e_ps = eps.tile([128, OUT], F32, tag="eps")
                e_tiles[st] = e_ps
                emit_mm(e_ps, 0, *xts[st])

            # rest of W + lookahead X
            emit_w(4)
            xts[3] = emit_x(3)
            emit_w(5)
            xts[4] = emit_x(4)
            emit_w(6)
            emit_w(7)

            # --- phase B: c2 bank 1, then bank-1 matmuls + epilogues -------
            emit_c2_bank(c2_ps0, 0, 1)
            finish_c2(c2_ps0, 0)

            def emit_softmax(bb):
                sl = slice(bb * ST_PER_B, (bb + 1) * ST_PER_B)
                m1 = smpool.tile([128, 1], F32, tag="m1")
                nc.vector.tensor_reduce(m1[:], att_all[:, sl],
                                        axis=mybir.AxisListType.X,
                                        op=ALU.max)
                mall = smpool.tile([128, 1], F32, tag="mall")
                nc.gpsimd.partition_all_reduce(mall[:], m1[:], channels=128,
                                               reduce_op=bass_isa.ReduceOp.max)
                nmall = smpool.tile([128, 1], F32, tag="nmall")
                nc.vector.tensor_scalar_mul(nmall[:], mall[:], -1.0)
                ex = smpool.tile([128, ST_PER_B], F32, tag="ex")
                rs = smpool.tile([128, 1], F32, tag="rs")
                nc.scalar.activation(ex[:], att_all[:, sl], AF.Exp,
                                     bias=nmall[:], accum_out=rs[:])
                tot = smpool.tile([128, 1], F32, tag="tot")
                nc.gpsimd.partition_all_reduce(tot[:], rs[:], channels=128,
                                               reduce_op=bass_isa.ReduceOp.add)
                rec = smpool.tile([128, 1], F32, tag="rec")
                nc.vector.reciprocal(rec[:], tot[:])
                res_t = smpool.tile([128, ST_PER_B], F32, tag="res")
                nc.vector.tensor_scalar_mul(res_t[:], ex[:], rec[:])
                nc.scalar.dma_start(
                    outd[bb].rearrange("(stl p) -> p stl", p=128), res_t[:])

            def emit_epilogue(st, e_ps):
                b_idx = st // ST_PER_B
                nc.vector.tensor_add(e_ps[:], e_ps[:], c2bc_sb[b_idx][:])
                relu_out = spool.tile([128, OUT], BF16, tag="relu")
                nc.vector.scalar_tensor_tensor(
                    relu_out[:], e_ps[:], 0.0, vbc[:],
                    op0=ALU.max, op1=ALU.mult,
                    accum_out=att_all[:, st:st + 1])
                if st % ST_PER_B == ST_PER_B - 1:
                    emit_softmax(st // ST_PER_B)

            for st in range(PHASE_TILES):
                emit_mm(e_tiles[st], 1, *xts[st])
                emit_epilogue(st, e_tiles[st])
                del e_tiles[st]

            # --- steady state ----------------------------------------------
            LOOKAHEAD = 2
            next_emit = 5            # s-tiles 0..4 emitted above
            for st in range(PHASE_TILES, N_ST):
                if st == 12:
                    # batch-1 c2, needed from s-tile 16 on
                    c2_ps1 = cps.tile([128, OUT], F32, tag="cps")
                    emit_c2_bank(c2_ps1, 1, 0)
                    emit_c2_bank(c2_ps1, 1, 1)
                    finish_c2(c2_ps1, 1)
                e_ps = eps.tile([128, OUT], F32, tag="eps")
                xTh, xTe = xts.pop(st)
                emit_mm(e_ps, 0, xTh, xTe)
                emit_mm(e_ps, 1, xTh, xTe)
                emit_epilogue(st, e_ps)
                while next_emit < N_ST and next_emit <= st + 1 + LOOKAHEAD:
                    xts[next_emit] = emit_x(next_emit)
                    next_emit += 1

    nc.compile()
    return nc


_NC = None


def _get_nc():
    global _NC
    if _NC is None:
        _NC = build_nc()
    return _NC


def _in_maps(hidden, encoder_outputs, c_t, W, b, v):
    hidden = np.ascontiguousarray(hidden, dtype=np.float32)
    encoder_outputs = np.ascontiguousarray(encoder_outputs, dtype=np.float32)
    c_t = np.ascontiguousarray(c_t, dtype=np.float32)
    W = np.ascontiguousarray(W, dtype=np.float32)
    b = np.ascontiguousarray(b, dtype=np.float32)
    v = np.ascontiguousarray(v, dtype=np.float32)
    maps = []
    for i in range(N_CORES):
        bs = slice(i * B_LOC, (i + 1) * B_LOC)
        maps.append({
            "hidden": hidden[bs].reshape(S_LOC, D),
            "enc": encoder_outputs[bs].reshape(S_LOC, D),
            "ct": c_t[bs],
            "W": W, "b": b, "v": v,
        })
    return maps


def run(hidden, encoder_outputs, c_t, W, b, v, trace=False, tmpdir=None):
    nc = _get_nc()
    maps = _in_maps(hidden, encoder_outputs, c_t, W, b, v)
    res = run_bass_kernel_spmd(nc, maps, list(range(N_CORES)), trace=trace,
                               tmpdir=tmpdir)
    out = np.concatenate([res.results[i]["out"] for i in range(N_CORES)],
                         axis=0)
    return out, res


def kernel(hidden, encoder_outputs, c_t, W, b, v):
    out, _ = run(hidden, encoder_outputs, c_t, W, b, v)
    return out


# revision 15
# speedup vs baseline: 1.0586x; 1.0586x over previous
"""Trainium2 Bass kernel for nn_Attention (sparse_attention variant).

Computes, for inputs hidden/encoder_outputs [B,S,D], c_t [B,D], W [OUT,3D],
b [OUT], v [OUT]:

    cat       = concat([hidden, broadcast(c_t), encoder_outputs], axis=2)
    energy    = relu(cat @ W.T + b)            # [B, S, OUT]
    attention = energy @ v                     # [B, S]
    out       = softmax(attention, axis=1)

Strategy (8 NeuronCores, data-parallel over batch, 2 batches/core):
  - Split W = [W1 | W2 | W3] over the feature axis; the c_t term is a
    rank-1 row c2[b,:] = c_t[b] @ W2.T + b added per batch, so the big
    contraction is only 2D (hidden + encoder halves).
  - All matmuls run in fp16 with fp32 PSUM accumulate.  The contraction
    dim f must sit on SBUF partitions, so both X and W are transposed
    on-chip with SBUF->SBUF xbar DMAs (InstDmaTransposeAnt): natural
    fp32 tiles are DMA'd in (contiguous), cast to fp16 (DVE for X,
    SWDGE cast-load for W), then one batched transpose per 128-row
    tile emits the [f, 8, s] chunked-transposed layout directly.  No
    DRAM scratch round-trip and no PE transposes.
  - Startup is bank-split: the first W row-chunks (o 0..511) unblock
    bank-0 matmuls + the bank-0 half of c2 while o 512..1023 is still
    loading, so the PE starts ~25-30us in instead of waiting for W.
  - Main loop per 128-row s-tile: accumulate pre[s, o] over 16 f-chunks
    x 2 PSUM banks; VectorE adds the broadcast c2 row and does a fused
    relu(pre)*v + row-sum (accum_out) -> attention logits.
  - Softmax over S=2048 per batch: 128x16 tile, DVE free-dim reduce +
    GpSimd partition all-reduce, ScalarE exp, DVE normalize.
"""

import sys
import numpy as np

for _p in ("/opt/trn_rl_repo",):
    if _p not in sys.path:
        sys.path.insert(0, _p)

import concourse.bass as bass
import concourse.bacc as bacc
import concourse.tile as tile
from concourse import mybir, bass_isa
from concourse.bass_utils import run_bass_kernel_spmd

F32 = mybir.dt.float32
F16 = mybir.dt.float16
BF16 = mybir.dt.bfloat16
AF = mybir.ActivationFunctionType
ALU = mybir.AluOpType

B, S, D, OUT = 16, 2048, 1024, 1024
N_CORES = 8
B_LOC = B // N_CORES            # batches per core
S_LOC = B_LOC * S               # 4096 rows of X per core
N_ST = S_LOC // 128             # 32 s-tiles per core
ST_PER_B = S // 128             # 16 s-tiles per batch
FC = D // 128                   # 8 feature chunks per tensor
NB = OUT // 512                 # 2 PSUM banks across OUT
OC = OUT // 128                 # 8 row-chunks of W
PHASE_TILES = 3                 # s-tiles that run bank0 before W fully loaded


def build_nc():
    nc = bacc.Bacc("TRN2", target_bir_lowering=False, debug=False,
                   num_devices=N_CORES, dynamic_dma_scratch_size=32768)

    hid = nc.dram_tensor("hidden", [S_LOC, D], F32, kind="ExternalInput").ap()
    enc = nc.dram_tensor("enc", [S_LOC, D], F32, kind="ExternalInput").ap()
    ct = nc.dram_tensor("ct", [B_LOC, D], F32, kind="ExternalInput").ap()
    Wd = nc.dram_tensor("W", [OUT, 3 * D], F32, kind="ExternalInput").ap()
    bd = nc.dram_tensor("b", [OUT], F32, kind="ExternalInput").ap()
    vd = nc.dram_tensor("v", [OUT], F32, kind="ExternalInput").ap()
    outd = nc.dram_tensor("out", [B_LOC, S], F32, kind="ExternalOutput").ap()

    with tile.TileContext(nc) as tc:
        with (
            tc.tile_pool(name="const", bufs=1) as cpool,
            tc.tile_pool(name="wT", bufs=1) as wpool,
            tc.tile_pool(name="w16", bufs=2) as w16p,
            tc.tile_pool(name="x16", bufs=1) as x16p,
            tc.tile_pool(name="xT", bufs=1) as xTp,
            tc.tile_pool(name="scratch", bufs=2) as spool,
            tc.tile_pool(name="sm", bufs=2) as smpool,
            tc.tile_pool(name="eps", bufs=3, space=bass.MemorySpace.PSUM) as eps,
            tc.tile_pool(name="cps", bufs=1, space=bass.MemorySpace.PSUM) as cps,
        ):
            ones_k1 = cpool.tile([1, 128], F16)
            nc.vector.memset(ones_k1[:], 1.0)
            att_all = cpool.tile([128, N_ST], F32)   # attention logits

            # ---- small constants ------------------------------------------
            v_f = cpool.tile([1, OUT], F32)
            nc.scalar.dma_start(v_f[:], vd[None, :])
            v_h = cpool.tile([1, OUT], F16)
            nc.vector.tensor_copy(v_h[:], v_f[:])
            b_f = cpool.tile([1, OUT], F32)
            nc.scalar.dma_start(b_f[:], bd[None, :])
            # ctT[p, fc, bb] = c_t[bb, fc*128+p]
            ctT_f = cpool.tile([128, FC, B_LOC], F32)
            for bb in range(B_LOC):
                nc.scalar.dma_start(ctT_f[:, :, bb],
                                    ct[bb].rearrange("(fc p) -> p fc", p=128))
            ctT_h = cpool.tile([128, FC, B_LOC], F16)
            nc.vector.tensor_copy(ctT_h[:], ctT_f[:])

            # vbc[p, o] = v[o] (fp16) for the fused relu*v epilogue
            vbc_ps = cps.tile([128, OUT], F32, tag="cps")
            for ob in range(NB):
                sl = slice(ob * 512, (ob + 1) * 512)
                nc.tensor.matmul(vbc_ps[:, sl], ones_k1[:], v_h[:, sl],
                                 start=True, stop=True)
            vbc = cpool.tile([128, OUT], F16)
            nc.vector.tensor_copy(vbc[:], vbc_ps[:])

            # ---- W pipeline: SWDGE cast-load + xbar transpose -------------
            # wT[p, j, o] = W[o, j*128+p]; j in [0,8)=W1, [8,16)=W2,
            # [16,24)=W3.
            wT = wpool.tile([128, 3 * FC, OUT], F16)

            # NOTE: all xbar transposes go on the sync ring only — two
            # transposed DMAs in flight on different queues corrupt each
            # other (verified on HW).  SWDGE cast-loads are batched into
            # multi-MB instructions: small SWDGE DMAs run ~4x slower.
            def emit_wgroup(g):
                # 4 W row-chunks in one SWDGE cast-load: [128, 4, 3*D]
                w16 = w16p.tile([128, 4, 3 * D], F16, tag="w16", bufs=1)
                src = Wd[g * 512:(g + 1) * 512, :].rearrange(
                    "(oc p) f -> p oc f", p=128)
                nc.gpsimd.dma_start(w16[:], src)
                for sub in range(4):
                    oc = g * 4 + sub
                    nc.sync.dma_start(wT[:, :, oc * 128:(oc + 1) * 128],
                                      w16[:, sub, :], transpose=True)

            # ---- X pipeline: batched SWDGE cast-load + xbar transposes ----
            def emit_xgroup(g, xts):
                # 4 s-tiles per group, one 2 MB cast-load per tensor
                x16h = x16p.tile([128, 4, D], F16, tag="xh16", bufs=2)
                nc.gpsimd.dma_start(x16h[:], hid[g * 512:(g + 1) * 512, :]
                                    .rearrange("(st p) f -> p st f", p=128))
                x16e = x16p.tile([128, 4, D], F16, tag="xe16", bufs=2)
                nc.gpsimd.dma_start(x16e[:], enc[g * 512:(g + 1) * 512, :]
                                    .rearrange("(st p) f -> p st f", p=128))
                for sub in range(4):
                    st = g * 4 + sub
                    xTh = xTp.tile([128, FC, 128], F16, tag="xTh", bufs=12)
                    nc.sync.dma_start(xTh[:], x16h[:, sub, :], transpose=True)
                    xTe = xTp.tile([128, FC, 128], F16, tag="xTe", bufs=12)
                    nc.sync.dma_start(xTe[:], x16e[:, sub, :], transpose=True)
                    xts[st] = (xTh, xTe)

            # W first (2 big loads), then the first two X groups.
            xts = {}
            emit_wgroup(0)
            emit_xgroup(0, xts)
            emit_wgroup(1)
            emit_xgroup(1, xts)

            def emit_mm(e_ps, ob, xTh, xTe):
                sl = slice(ob * 512, (ob + 1) * 512)
                for fc in range(FC):
                    nc.tensor.matmul(e_ps[:, sl], xTh[:, fc, :],
                                     wT[:, fc, sl],
                                     start=(fc == 0), stop=False)
                for fc in range(FC):
                    nc.tensor.matmul(e_ps[:, sl], xTe[:, fc, :],
                                     wT[:, 2 * FC + fc, sl],
                                     start=False, stop=(fc == FC - 1))

            # c2[b, o] = c_t[b] @ W2.T + b, one PSUM row per batch
            c2bc_sb = {}

            def emit_c2_bank(c2_ps, bb, ob):
                sl = slice(ob * 512, (ob + 1) * 512)
                for fc in range(FC):
                    nc.tensor.matmul(c2_ps[:1, sl], ctT_h[:, fc, bb:bb + 1],
                                     wT[:, FC + fc, sl],
                                     start=(fc == 0), stop=(fc == FC - 1))

            def finish_c2(c2_ps, bb):
                c2b = cpool.tile([1, OUT], F16, tag=f"c2b_{bb}")
                nc.vector.tensor_add(c2b[:], c2_ps[:1, :], b_f[:])
                c2bc_ps = cps.tile([128, OUT], F32, tag="cps")
                for ob in range(NB):
                    sl = slice(ob * 512, (ob + 1) * 512)
                    nc.tensor.matmul(c2bc_ps[:, sl], ones_k1[:],
                                     c2b[:, sl], start=True, stop=True)
                c2bc = cpool.tile([128, OUT], F16, tag=f"c2bc_{bb}")
                nc.vector.tensor_copy(c2bc[:], c2bc_ps[:])
                c2bc_sb[bb] = c2bc

            # --- phase A: bank 0 only (W rows 0..511 suffice) --------------
            c2_ps0 = cps.tile([128, OUT], F32, tag="cps")
            emit_c2_bank(c2_ps0, 0, 0)
            e_tiles = {}
            for st in range(PHASE_TILES):
                e_ps = eps.tile([128, OUT], F32, tag="eps")
                e_tiles[st] = e_ps
                emit_mm(e_ps, 0, *xts[st])

            # --- phase B: c2 bank 1, then bank-1 matmuls + epilogues -------
            emit_c2_bank(c2_ps0, 0, 1)
            finish_c2(c2_ps0, 0)

            def emit_softmax(bb):
                sl = slice(bb * ST_PER_B, (bb + 1) * ST_PER_B)
                m1 = smpool.tile([128, 1], F32, tag="m1")
                nc.vector.tensor_reduce(m1[:], att_all[:, sl],
                                        axis=mybir.AxisListType.X,
                                        op=ALU.max)
                mall = smpool.tile([128, 1], F32, tag="mall")
                nc.gpsimd.partition_all_reduce(mall[:], m1[:], channels=128,
                                               reduce_op=bass_isa.ReduceOp.max)
                nmall = smpool.tile([128, 1], F32, tag="nmall")
                nc.vector.tensor_scalar_mul(nmall[:], mall[:], -1.0)
                ex = smpool.tile([128, ST_PER_B], F32, tag="ex")
                rs = smpool.tile([128, 1], F32, tag="rs")
                nc.scalar.activation(ex[:], att_all[:, sl], AF.Exp,
                                     bias=nmall[:], accum_out=rs[:])
                tot = smpool.tile([128, 1], F32, tag="tot")
                nc.gpsimd.partition_all_reduce(tot[:], rs[:], channels=128,
                                               reduce_op=bass_isa.ReduceOp.add)
                rec = smpool.tile([128, 1], F32, tag="rec")
                nc.vector.reciprocal(rec[:], tot[:])
                res_t = smpool.tile([128, ST_PER_B], F32, tag="res")
                nc.vector.tensor_scalar_mul(res_t[:], ex[:], rec[:])
                nc.scalar.dma_start(
                    outd[bb].rearrange("(stl p) -> p stl", p=128), res_t[:])

            def emit_epilogue(st, e_ps):
                b_idx = st // ST_PER_B
                nc.vector.tensor_add(e_ps[:], e_ps[:], c2bc_sb[b_idx][:])
                relu_out = spool.tile([128, OUT], BF16, tag="relu")
                nc.vector.scalar_tensor_tensor(
                    relu_out[:], e_ps[:], 0.0, vbc[:],
                    op0=ALU.max, op1=ALU.mult,
                    accum_out=att_all[:, st:st + 1])
                if st % ST_PER_B == ST_PER_B - 1:
                    emit_softmax(st // ST_PER_B)

            for st in range(PHASE_TILES):
                emit_mm(e_tiles[st], 1, *xts[st])
                emit_epilogue(st, e_tiles[st])
                del e_tiles[st]

            # --- steady state ----------------------------------------------
            N_G = N_ST // 4
            for st in range(PHASE_TILES, N_ST):
                if st == 12:
                    # batch-1 c2, needed from s-tile 16 on
                    c2_ps1 = cps.tile([128, OUT], F32, tag="cps")
                    emit_c2_bank(c2_ps1, 1, 0)
                    emit_c2_bank(c2_ps1, 1, 1)
                    finish_c2(c2_ps1, 1)
                e_ps = eps.tile([128, OUT], F32, tag="eps")
                xTh, xTe = xts.pop(st)
                emit_mm(e_ps, 0, xTh, xTe)
                emit_mm(e_ps, 1, xTh, xTe)
                emit_epilogue(st, e_ps)
                if (st + 1) % 4 == 0:
                    g = (st + 1) // 4 + 1
                    if g < N_G:
                        emit_xgroup(g, xts)

    nc.compile()
    return nc


_NC = None


def _get_nc():
    global _NC
    if _NC is None:
        _NC = build_nc()
    return _NC


def _in_maps(hidden, encoder_outputs, c_t, W, b, v):
    hidden = np.ascontiguousarray(hidden, dtype=np.float32)
    encoder_outputs = np.ascontiguousarray(encoder_outputs, dtype=np.float32)
    c_t = np.ascontiguousarray(c_t, dtype=np.float32)
    W = np.ascontiguousarray(W, dtype=np.float32)
    b = np.ascontiguousarray(b, dtype=np.float32)
    v = np.ascontiguousarray(v, dtype=np.float32)
    maps = []
    for i in range(N_CORES):
        bs = slice(i * B_LOC, (i + 1) * B_LOC)
        maps.append({
            "hidden": hidden[bs].reshape(S_LOC, D),
            "enc": encoder_outputs[bs].reshape(S_LOC, D),
            "ct": c_t[bs],
            "W": W, "b": b, "v": v,
        })
    return maps


def run(hidden, encoder_outputs, c_t, W, b, v, trace=False, tmpdir=None):
    nc = _get_nc()
    maps = _in_maps(hidden, encoder_outputs, c_t, W, b, v)
    res = run_bass_kernel_spmd(nc, maps, list(range(N_CORES)), trace=trace,
                               tmpdir=tmpdir)
    out = np.concatenate([res.results[i]["out"] for i in range(N_CORES)],
                         axis=0)
    return out, res


def kernel(hidden, encoder_outputs, c_t, W, b, v):
    out, _ = run(hidden, encoder_outputs, c_t, W, b, v)
    return out
